# revision 14
# baseline (speedup 1.0000x reference)
"""Grouped-query attention (B=2, T=2048, d_model=2048, 32 Q heads / 8 KV heads)
sharded over 8 NeuronCores: batch x head-block tensor parallel.

Core c handles batch b = c//4 and head-block hb = c%4 (8 q heads = 2 kv groups).
bf16 matmul operands everywhere (fp32 PSUM accumulate); host feeds pre-transposed
bf16 inputs and sums/transposes per-core partials.

v2 structure:
  - phase A (projections) rolled over 512-token blocks (hardware loop, 4 iters)
  - phase B fully unrolled over 8 query blocks j of 256; per j scans exactly
    the causal prefix of key tiles (NKT = 2j+2). Score matmuls for the two kv
    groups auto-row-tile ((0,0) and (64,0) 64x128 PE tiles run concurrently --
    operands stay at their natural partitions, no staging DMAs). V_aug carries
    a 64-wide ones block so AV matmuls have M=128 and rows 64-127 of og hold
    the softmax denominator replicated across 64 partitions (fast vectorized
    reciprocal, no gpsimd broadcast). Causal masking: multiplicative 0/1 bf16
    mask on P after exp, applied only to the two diagonal key tiles of each j.
  - phase C (output projection) rolled over 1024-token halves (2 iters)
"""

import numpy as np

D_MODEL = 2048
T = 2048
B = 2
DK = 64
NREP = 4

_CACHE: dict = {}


# --------------------------------------------------------------------------
# device kernel
# --------------------------------------------------------------------------
def _build_nc(loop_n=1, unroll=False):
    import concourse.bass as bass
    import concourse.mybir as mybir
    import concourse.tile as tile
    from concourse import bacc
    from concourse.masks import make_identity

    F32 = mybir.dt.float32
    BF16 = mybir.dt.bfloat16
    EXP = mybir.ActivationFunctionType.Exp
    ds = bass.ds

    nc = bacc.Bacc("TRN2", target_bir_lowering=False, debug=False)

    xT = nc.dram_tensor("xT", [128, 16 * 2048], BF16, kind="ExternalInput").ap()
    WqT = nc.dram_tensor("WqT", [128, 16 * 512], BF16, kind="ExternalInput").ap()
    WkT = nc.dram_tensor("WkT", [128, 16 * 128], BF16, kind="ExternalInput").ap()
    WvT = nc.dram_tensor("WvT", [128, 16 * 128], BF16, kind="ExternalInput").ap()
    WoT = nc.dram_tensor("WoT", [128, 4 * 2048], BF16, kind="ExternalInput").ap()
    MSK4 = nc.dram_tensor("MSK4", [128, 1024], BF16, kind="ExternalInput").ap()
    YT = nc.dram_tensor("YT", [2048, 2048], BF16, kind="ExternalOutput").ap()

    with tile.TileContext(nc) as tc:
        def loop(n, body, **kw):
            """Hardware For_i over range(n), or python-unrolled (for the
            timeline simulator, which can't resolve reg-mode branches)."""
            if unroll:
                for v in range(n):
                    body(v)
            else:
                with tc.For_i(0, n, 1, **kw) as v:
                    body(v)

        with tc.tile_pool(name="consts", bufs=1) as consts, \
             tc.tile_pool(name="wts", bufs=1) as wts, \
             tc.tile_pool(name="persist", bufs=1) as persist, \
             tc.tile_pool(name="pp", bufs=4) as ppool, \
             tc.tile_pool(name="rcp", bufs=2) as rcp, \
             tc.tile_pool(name="ytp", bufs=2) as ytp, \
             tc.tile_pool(name="ps", bufs=1, space="PSUM") as ps:

            # ---------------- constants (outside the timing loop) ----------
            idl_f32 = consts.tile([128, 128], F32)
            make_identity(nc, idl_f32)
            idl = consts.tile([128, 128], BF16)
            nc.vector.tensor_copy(idl, idl_f32)

            x_sb = wts.tile([128, 16 * 2048], BF16)   # col = ck*2048 + tok
            wq_sb = wts.tile([128, 16 * 512], BF16)   # col = ck*512 + qout
            wk_sb = wts.tile([128, 16 * 128], BF16)   # col = ck*128 + kout
            wv_sb = wts.tile([128, 16 * 128], BF16)
            wo_sb = wts.tile([128, 4 * 2048], BF16)   # col = ic*2048 + out
            msk_sb = consts.tile([128, 1024], BF16)   # 4 x [128,256] mask blocks

            qt_sb = persist.tile([128, 8192], BF16)   # col = j*1024 + r*256 + qi
            kt_sb = persist.tile([128, 2048], BF16)   # [kvd, tok]
            vt_sb = persist.tile([128, 2048], BF16)   # [kvd, tok]
            va_sb = persist.tile([128, 4096], BF16)   # 32 x [128 tok, 64 v | 64 ones]
            otn_sb = persist.tile([128, 8192], BF16)  # col = oc*2048 + tok
            otn2_sb = persist.tile([64, 8192], BF16)  # odd-rep rows, staged up

            # ones block of every V_aug tile (values never change)
            ones_ap = bass.AP(tensor=va_sb.tensor, offset=va_sb.offset + 64,
                              ap=[va_sb.ap[0], [128, 32], [1, 64]])
            nc.vector.memset(ones_ap, 1.0)

            def phase_a(tb):
                qa = [ps.tile([128, 1024], F32, tag="st", bufs=2,
                              name=f"qa{qp}") for qp in range(2)]
                kv = ps.tile([128, 1024], F32, tag="og0", bufs=1, name="kv")
                for ck in range(16):
                    xs = x_sb[:, ds(tb * 512 + ck * 2048, 512)]
                    first, last = ck == 0, ck == 15
                    for qp in range(2):
                        for u in range(2):
                            qc = 2 * qp + u
                            nc.tensor.matmul(
                                qa[qp][:, u * 512:(u + 1) * 512],
                                wq_sb[:, ck * 512 + qc * 128:
                                      ck * 512 + (qc + 1) * 128],
                                xs, start=first, stop=last)
                    nc.tensor.matmul(
                        kv[:, 0:512], wk_sb[:, ck * 128:(ck + 1) * 128],
                        xs, start=first, stop=last)
                    nc.tensor.matmul(
                        kv[:, 512:1024], wv_sb[:, ck * 128:(ck + 1) * 128],
                        xs, start=first, stop=last)
                # evacuate: qt_sb col = j*1024 + qc*256 + qi  (j = tok//256)
                for qp in range(2):
                    for jh in range(2):
                        src = bass.AP(
                            tensor=qa[qp].tensor,
                            offset=qa[qp].offset + jh * 256,
                            ap=[qa[qp].ap[0], [512, 2], [1, 256]])
                        nc.vector.tensor_copy(
                            qt_sb[:, ds(tb * 2048 + jh * 1024 + qp * 512,
                                        512)], src)
                nc.vector.tensor_copy(
                    kt_sb[:, ds(tb * 512, 512)], kv[:, 0:512])
                nc.vector.tensor_copy(
                    vt_sb[:, ds(tb * 512, 512)], kv[:, 512:1024])
                # V_aug build for this tb's 4 key tiles: transpose VT
                # 128-blocks into [tok, vdim] tiles
                for i in range(4):
                    vtp = ps.tile([128, 128], BF16, tag="og1", bufs=1,
                                  name=f"vtp{i}")
                    nc.tensor.transpose(
                        vtp, vt_sb[:, ds(tb * 512 + i * 128, 128)], idl)
                    # vtp cols 0-63 = g0 vdims -> va tile kt; 64-127 = g1
                    # vdims -> va tile 16+kt (ones block at +64 untouched)
                    dest = bass.AP(tensor=va_sb.tensor,
                                   offset=va_sb.offset + tb * 512 + i * 128,
                                   ap=[va_sb.ap[0], [16 * 128, 2], [1, 64]])
                    src = bass.AP(tensor=vtp.tensor, offset=vtp.offset,
                                  ap=[vtp.ap[0], [64, 2], [1, 64]])
                    nc.vector.tensor_copy(dest, src)

            def phase_b(j):
                NKT = 2 * j + 2
                og = [ps.tile([128, 1024], F32, tag=f"og{g}", bufs=1,
                              name=f"og{j}_{g}") for g in range(2)]
                prev_p = None

                def av1(pkt, g, p):
                    for h in range(2):
                        nc.tensor.matmul(
                            og[g][:, h * 512:(h + 1) * 512],
                            va_sb[:, (g * 16 + pkt) * 128:
                                  (g * 16 + pkt) * 128 + 128],
                            p[:, h * 512:(h + 1) * 512],
                            start=(pkt == 0), stop=(pkt == NKT - 1))

                # key tiles in pairs: a pair's 8 score matmuls all run in
                # 64x128 row-tiled mode back-to-back, then the previous
                # pair's 8 AV matmuls in 128x128 mode -- one PE tiling-mode
                # round trip per pair instead of per tile
                for kp in range(NKT // 2):
                    cur_p = []
                    for kt in (2 * kp, 2 * kp + 1):
                        for g in range(2):
                            # scores: 64x128 row tile (0,0)/(64,0) per group
                            # -- operands at the group's natural partitions,
                            # both groups stream through the PE concurrently
                            st = ps.tile([128, 1024], F32, tag="st", bufs=2,
                                         name=f"st{j}_{kt}_{g}")
                            lhsT = kt_sb[64 * g:64 * (g + 1),
                                         kt * 128:(kt + 1) * 128]
                            for h in range(2):
                                nc.tensor.matmul(
                                    st[:, h * 512:(h + 1) * 512], lhsT,
                                    qt_sb[64 * g:64 * (g + 1),
                                          ds(j * 1024 + h * 512, 512)],
                                    start=True, stop=True)
                            p = ppool.tile([128, 1024], BF16, tag="p",
                                           name=f"p{j}_{kt}_{g}")
                            nc.scalar.activation(p, st, EXP, scale=0.125)
                            # causal mask: only the two diagonal key tiles
                            # need it (kt < 2j is strictly below the diagonal)
                            if kt >= 2 * j:
                                moff = 256 * (1 + kt - 2 * j)
                                mask_b = bass.AP(
                                    tensor=msk_sb.tensor,
                                    offset=msk_sb.offset + moff,
                                    ap=[msk_sb.ap[0], [0, 4], [1, 256]])
                                nc.vector.tensor_mul(p, p, mask_b)
                            cur_p.append((kt, g, p))
                    # software pipeline: AV for the PREVIOUS pair runs while
                    # this pair's mask+exp are in flight
                    if prev_p is not None:
                        for pkt, g, p in prev_p:
                            av1(pkt, g, p)
                    prev_p = cur_p
                for pkt, g, p in prev_p:
                    av1(pkt, g, p)
                # normalize + evacuate to otn; og rows 64-127 hold the
                # denominator replicated across 64 partitions
                for g in range(2):
                    rec = rcp.tile([64, 1024], F32, tag="rec", name=f"r{g}")
                    nc.vector.reciprocal(rec, og[g][64:128, :])
                    for r in range(4):
                        oc = 2 * g + r // 2
                        dst = otn_sb if r % 2 == 0 else otn2_sb
                        nc.vector.tensor_mul(
                            dst[0:64, ds(oc * 2048 + j * 256, 256)],
                            og[g][0:64, r * 256:(r + 1) * 256],
                            rec[:, r * 256:(r + 1) * 256])

            def phase_c(th):
                for oc in range(16):
                    yt = ps.tile([128, 1024], F32, tag="st", bufs=2,
                                 name=f"yt{oc}")
                    for ic in range(4):
                        for h in range(2):
                            nc.tensor.matmul(
                                yt[:, h * 512:(h + 1) * 512],
                                wo_sb[:, ic * 2048 + oc * 128:
                                      ic * 2048 + (oc + 1) * 128],
                                otn_sb[:, ds(ic * 2048 + th * 1024
                                             + h * 512, 512)],
                                start=(ic == 0), stop=(ic == 3))
                    yt_sb = ytp.tile([128, 1024], BF16, tag="ytsb",
                                     name=f"ytsb{oc}")
                    nc.vector.tensor_copy(yt_sb, yt)
                    nc.sync.dma_start(
                        out=YT[oc * 128:(oc + 1) * 128, ds(th * 1024,
                                                           1024)],
                        in_=yt_sb)

            def body(_rep):
                # ---------------- input DMA ----------------
                # host pre-arranges every input into its SBUF layout, so each
                # load is a plain contiguous 2D DMA (big descriptors, no
                # scatter). x/wq/wk/wv stream in 4 ck-groups so phase A can
                # start consuming group 0 while later groups are in flight;
                # wo is only needed by phase C, load it last.
                nc.sync.dma_start(out=msk_sb, in_=MSK4)
                for cg in range(4):
                    nc.sync.dma_start(out=x_sb[:, ds(cg * 8192, 8192)],
                                      in_=xT[:, ds(cg * 8192, 8192)])
                    nc.sync.dma_start(out=wq_sb[:, ds(cg * 2048, 2048)],
                                      in_=WqT[:, ds(cg * 2048, 2048)])
                    nc.sync.dma_start(out=wk_sb[:, ds(cg * 512, 512)],
                                      in_=WkT[:, ds(cg * 512, 512)])
                    nc.sync.dma_start(out=wv_sb[:, ds(cg * 512, 512)],
                                      in_=WvT[:, ds(cg * 512, 512)])
                nc.sync.dma_start(out=wo_sb, in_=WoT)

                # ---------------- phase A: projections (unrolled over tb;
                # transpose lhsT offsets must be compile-time constants) -----
                for tb in range(4):
                    phase_a(tb)

                # ---------------- phase B: attention (unrolled over j) ------
                for j in range(8):
                    phase_b(j)

                # stage odd-rep otn rows up to partitions 64-127
                nc.sync.dma_start(out=otn_sb[64:128, :], in_=otn2_sb)

                # ---------------- phase C: output projection (rolled th) ----
                loop(2, phase_c)

            loop(loop_n, body)

    nc.compile()
    return nc


def _get_nc():
    if "nc" not in _CACHE:
        _CACHE["nc"] = _build_nc()
    return _CACHE["nc"]


# --------------------------------------------------------------------------
# host wrapper
# --------------------------------------------------------------------------
def _bf16(a):
    import ml_dtypes
    return np.ascontiguousarray(np.asarray(a).astype(ml_dtypes.bfloat16))


def _make_mask() -> np.ndarray:
    """4 multiplicative 0/1 blocks of [128, 256] (broadcast over reps):
    block 0: all-pass; 1: diag kt==2j; 2: diag kt==2j+1; 3: all-blocked."""
    ki = np.arange(128)[:, None]
    qi = np.arange(256)[None, :]
    o = np.ones((128, 256), np.float32)
    m0 = np.where(ki <= qi, 1.0, 0.0).astype(np.float32)
    m1 = np.where(128 + ki <= qi, 1.0, 0.0).astype(np.float32)
    mf = np.zeros((128, 256), np.float32)
    return np.concatenate([o, m0, m1, mf], axis=1)  # [128, 1024]


def _core_inputs(x, Wq, Wk, Wv, Wo, c, mask):
    b, hb = c // 4, c % 4
    xT_c = np.ascontiguousarray(x[b].T)
    # interleave q heads: chunk qc = [g0 rep qc (64) | g1 rep qc (64)]
    g0, g1 = 2 * hb, 2 * hb + 1
    cols = []
    for qc in range(NREP):
        cols.append(Wq[g0 * 256 + qc * 64: g0 * 256 + (qc + 1) * 64])
        cols.append(Wq[g1 * 256 + qc * 64: g1 * 256 + (qc + 1) * 64])
    WqT_c = np.ascontiguousarray(np.concatenate(cols, axis=0).T)
    WkT_c = np.ascontiguousarray(Wk[128 * hb:128 * (hb + 1)].T)
    WvT_c = np.ascontiguousarray(Wv[128 * hb:128 * (hb + 1)].T)
    WoT_c = np.ascontiguousarray(Wo[:, 512 * hb:512 * (hb + 1)].T)
    def _sb(a, nchunk):    # [nchunk*128, w] -> [128, nchunk*w] (ck-major cols)
        n = a.shape[0] // 128
        assert n == nchunk
        return a.reshape(n, 128, a.shape[1]).transpose(1, 0, 2).reshape(
            128, n * a.shape[1])
    return {"xT": _bf16(_sb(xT_c, 16)), "WqT": _bf16(_sb(WqT_c, 16)),
            "WkT": _bf16(_sb(WkT_c, 16)), "WvT": _bf16(_sb(WvT_c, 16)),
            "WoT": _bf16(_sb(WoT_c, 4)), "MSK4": _bf16(mask)}


def kernel(x, Wq, Wk, Wv, Wo, _trace=False, _trace_kwargs=None):
    from concourse import bass_utils

    x = np.asarray(x, dtype=np.float32)
    Wq = np.asarray(Wq, dtype=np.float32)
    Wk = np.asarray(Wk, dtype=np.float32)
    Wv = np.asarray(Wv, dtype=np.float32)
    Wo = np.asarray(Wo, dtype=np.float32)

    nc = _get_nc()
    mask = _make_mask()
    in_maps = [_core_inputs(x, Wq, Wk, Wv, Wo, c, mask) for c in range(8)]

    res = None
    last_exc = None
    for _attempt in range(3):
        try:
            res = bass_utils.run_bass_kernel_spmd(
                nc, in_maps, core_ids=list(range(8)),
                trace=_trace, **(_trace_kwargs or {}))
            break
        except Exception as e:  # transient device wedges happen; retry
            last_exc = e
    if res is None:
        raise last_exc

    Y = np.zeros((B, T, D_MODEL), dtype=np.float32)
    for c in range(8):
        Y[c // 4] += res.results[c]["YT"].T.astype(np.float32)
    if _trace:
        _CACHE["last_result"] = res
    return Y


# revision 24
# speedup vs baseline: 1.0335x; 1.0335x over previous
"""Grouped-query attention (B=2, T=2048, d_model=2048, 32 Q heads / 8 KV heads)
sharded over 8 NeuronCores: batch x head-block tensor parallel.

Core c handles batch b = c//4 and head-block hb = c%4 (8 q heads = 2 kv groups).
bf16 matmul operands everywhere (fp32 PSUM accumulate); host feeds pre-transposed
bf16 inputs and sums/transposes per-core partials.

v5: fully software-pipelined A/B/C on 1-bank PSUM tiles.
  - All PSUM accumulators are [128, 512] (one bank), so projections (A),
    attention (B) and output projection (C) can be co-resident in the 8 banks
    and overlap: A(tb) interleaves with B(2tb-2, 2tb-1); C's first token half
    fills PE slack inside B(6,7)'s exp-bound stretch; C's second half is the
    tail. B is ACT(exp)-bound, A/C are pure PE, so the stagger hides most of
    the exp cost.
  - Phase B processes query blocks j of 256 rows x 2 rep-halves; per half it
    scans exactly the causal prefix of key tiles (NKT = 2j+2) in batches of
    up to 4 (one PE tiling-mode round trip per batch). Score matmuls for the
    two kv groups auto-row-tile ((0,0)/(64,0) 64x128 PE tiles, concurrent).
    V_aug carries a 64-wide ones block so AV matmuls have M=128 and og rows
    64-127 hold the softmax denominator replicated across 64 partitions
    (vectorized reciprocal, no gpsimd broadcast). Causal masking:
    multiplicative 0/1 bf16 mask on P after exp, diagonal key tiles only.
  - Inputs stream in 4 ck-groups (phase A consumes group 0 while the rest
    are in flight); Wo loads last (phase C only).
"""

import numpy as np

D_MODEL = 2048
T = 2048
B = 2
DK = 64
NREP = 4

_CACHE: dict = {}


# --------------------------------------------------------------------------
# device kernel
# --------------------------------------------------------------------------
def _build_nc(loop_n=1, unroll=False):
    import concourse.bass as bass
    import concourse.mybir as mybir
    import concourse.tile as tile
    from concourse import bacc
    from concourse.masks import make_identity

    F32 = mybir.dt.float32
    BF16 = mybir.dt.bfloat16
    EXP = mybir.ActivationFunctionType.Exp
    ds = bass.ds

    nc = bacc.Bacc("TRN2", target_bir_lowering=False, debug=False)

    xT = nc.dram_tensor("xT", [128, 16 * 2048], BF16, kind="ExternalInput").ap()
    WqT = nc.dram_tensor("WqT", [128, 16 * 512], BF16, kind="ExternalInput").ap()
    WkT = nc.dram_tensor("WkT", [128, 16 * 128], BF16, kind="ExternalInput").ap()
    WvT = nc.dram_tensor("WvT", [128, 16 * 128], BF16, kind="ExternalInput").ap()
    WoT = nc.dram_tensor("WoT", [128, 4 * 2048], BF16, kind="ExternalInput").ap()
    MSK4 = nc.dram_tensor("MSK4", [128, 1024], BF16, kind="ExternalInput").ap()
    YT = nc.dram_tensor("YT", [2048, 2048], BF16, kind="ExternalOutput").ap()

    with tile.TileContext(nc) as tc:
        def loop(n, body, **kw):
            """Hardware For_i over range(n), or python-unrolled (for the
            timeline simulator, which can't resolve reg-mode branches)."""
            if unroll:
                for v in range(n):
                    body(v)
            else:
                with tc.For_i(0, n, 1, **kw) as v:
                    body(v)

        with tc.tile_pool(name="consts", bufs=1) as consts, \
             tc.tile_pool(name="wts", bufs=1) as wts, \
             tc.tile_pool(name="persist", bufs=1) as persist, \
             tc.tile_pool(name="pp", bufs=8) as ppool, \
             tc.tile_pool(name="rcp", bufs=4) as rcp, \
             tc.tile_pool(name="ytp", bufs=6) as ytp, \
             tc.tile_pool(name="ps", bufs=1, space="PSUM") as ps:

            # ---------------- constants (outside the timing loop) ----------
            idl_f32 = consts.tile([128, 128], F32)
            make_identity(nc, idl_f32)
            idl = consts.tile([128, 128], BF16)
            nc.vector.tensor_copy(idl, idl_f32)

            x_sb = wts.tile([128, 16 * 2048], BF16)   # col = ck*2048 + tok
            wq_sb = wts.tile([128, 16 * 512], BF16)   # col = ck*512 + qout
            wk_sb = wts.tile([128, 16 * 128], BF16)   # col = ck*128 + kout
            wv_sb = wts.tile([128, 16 * 128], BF16)
            wo_sb = wts.tile([128, 4 * 2048], BF16)   # col = ic*2048 + out
            msk_sb = consts.tile([128, 1024], BF16)   # 4 x [128,256] mask blocks

            qt_sb = persist.tile([128, 8192], BF16)   # col = j*1024 + r*256 + qi
            kt_sb = persist.tile([128, 2048], BF16)   # [kvd, tok]
            vt_sb = persist.tile([128, 2048], BF16)   # [kvd, tok]
            va_sb = persist.tile([128, 4096], BF16)   # 32 x [128 tok, 64 v | 64 ones]
            otn_sb = persist.tile([128, 8192], BF16)  # col = oc*2048 + tok
            otn2_sb = persist.tile([64, 8192], BF16)  # odd-rep rows, staged up

            # ones block of every V_aug tile (values never change)
            ones_ap = bass.AP(tensor=va_sb.tensor, offset=va_sb.offset + 64,
                              ap=[va_sb.ap[0], [128, 32], [1, 64]])
            nc.vector.memset(ones_ap, 1.0)

            def _a_evac(tb, grp, acc):
                if grp < 4:      # q chunk: qt_sb col = j*1024 + qc*256 + qi
                    dst = bass.AP(
                        tensor=qt_sb.tensor,
                        offset=qt_sb.offset + tb * 2048 + grp * 256,
                        ap=[qt_sb.ap[0], [1024, 2], [1, 256]])
                    nc.vector.tensor_copy(dst, acc)
                else:
                    dst_t = kt_sb if grp == 4 else vt_sb
                    nc.vector.tensor_copy(dst_t[:, ds(tb * 512, 512)], acc)

            def _a_mm(tb, grp, acc, ck):
                if grp < 4:
                    w = wq_sb[:, ck * 512 + grp * 128:
                              ck * 512 + (grp + 1) * 128]
                else:
                    w_t = wk_sb if grp == 4 else wv_sb
                    w = w_t[:, ck * 128:(ck + 1) * 128]
                nc.tensor.matmul(
                    acc, w, x_sb[:, ds(tb * 512 + ck * 2048, 512)],
                    start=(ck == 0), stop=(ck == 15))

            def phase_a(tb):
                # 6 serialized 1-bank accumulation groups: 4 q chunks, K, V
                # (runs concurrently with phase B, which owns st/og banks)
                for grp in range(6):
                    acc = ps.tile([128, 512], F32, tag="pa", bufs=2,
                                  name=f"pa{tb}_{grp}")
                    for ck in range(16):
                        _a_mm(tb, grp, acc, ck)
                    _a_evac(tb, grp, acc)
                # V_aug build for this tb's 4 key tiles: transpose VT
                # 128-blocks into [tok, vdim] tiles
                for i in range(4):
                    kt = 4 * tb + i
                    vtp = ps.tile([128, 128], BF16, tag="pa", bufs=2,
                                  name=f"vtp{kt}")
                    nc.tensor.transpose(
                        vtp, vt_sb[:, kt * 128:(kt + 1) * 128], idl)
                    # vtp cols 0-63 = g0 vdims -> va tile kt; 64-127 = g1
                    # vdims -> va tile 16+kt (ones block at +64 untouched)
                    dest = bass.AP(tensor=va_sb.tensor,
                                   offset=va_sb.offset + kt * 128,
                                   ap=[va_sb.ap[0], [16 * 128, 2], [1, 64]])
                    src = bass.AP(tensor=vtp.tensor, offset=vtp.offset,
                                  ap=[vtp.ap[0], [64, 2], [1, 64]])
                    nc.vector.tensor_copy(dest, src)

            def phase_a0():
                # ck-major variant for tb=0: all 6 accumulation groups in
                # flight (B hasn't started; its st/og banks are free), so the
                # PE tracks the input DMA stream instead of stalling on two
                # serialized groups.
                tags = ["pa", "pa", "st", "st", "og", "og"]
                tbufs = {"pa": 2, "st": 2, "og": 4}
                accs = [ps.tile([128, 512], F32, tag=tags[grp],
                                bufs=tbufs[tags[grp]],
                                name=f"pa0_{grp}") for grp in range(6)]
                for ck in range(16):
                    for grp in range(6):
                        _a_mm(0, grp, accs[grp], ck)
                for grp in range(6):
                    _a_evac(0, grp, accs[grp])
                for i in range(4):
                    vtp = ps.tile([128, 128], BF16, tag="pa", bufs=2,
                                  name=f"vtp0_{i}")
                    nc.tensor.transpose(
                        vtp, vt_sb[:, i * 128:(i + 1) * 128], idl)
                    dest = bass.AP(tensor=va_sb.tensor,
                                   offset=va_sb.offset + i * 128,
                                   ap=[va_sb.ap[0], [16 * 128, 2], [1, 64]])
                    src = bass.AP(tensor=vtp.tensor, offset=vtp.offset,
                                  ap=[vtp.ap[0], [64, 2], [1, 64]])
                    nc.vector.tensor_copy(dest, src)

            def _batches(nkt):
                """Key-tile batches of 4 (one tiling-mode round trip each)."""
                out, kt = [], 0
                while kt < nkt:
                    n = min(4, nkt - kt)
                    out.append(range(kt, kt + n))
                    kt += n
                return out

            def phase_b(j, filler=None):
                NKT = 2 * j + 2
                for hf in range(2):     # rep-halves of the 1024-wide q block
                    og = [ps.tile([128, 512], F32, tag="og", bufs=4,
                                  name=f"og{j}_{hf}_{g}") for g in range(2)]

                    def av1(pkt, g, p):
                        nc.tensor.matmul(
                            og[g],
                            va_sb[:, (g * 16 + pkt) * 128:
                                  (g * 16 + pkt) * 128 + 128],
                            p, start=(pkt == 0), stop=(pkt == NKT - 1))

                    prev = None
                    for batch in _batches(NKT):
                        cur = []
                        for kt in batch:
                            for g in range(2):
                                # scores: 64x128 row tiles (0,0)/(64,0); the
                                # two groups stream concurrently
                                st = ps.tile([128, 512], F32, tag="st",
                                             bufs=2, name=f"st{j}{hf}{kt}{g}")
                                nc.tensor.matmul(
                                    st,
                                    kt_sb[64 * g:64 * (g + 1),
                                          kt * 128:(kt + 1) * 128],
                                    qt_sb[64 * g:64 * (g + 1),
                                          ds(j * 1024 + hf * 512, 512)],
                                    start=True, stop=True)
                                p = ppool.tile([128, 512], BF16, tag="p",
                                               name=f"p{j}{hf}{kt}{g}")
                                nc.scalar.activation(p, st, EXP, scale=0.125)
                                # causal mask: diagonal key tiles only
                                if kt >= 2 * j:
                                    moff = 256 * (1 + kt - 2 * j)
                                    mask_b = bass.AP(
                                        tensor=msk_sb.tensor,
                                        offset=msk_sb.offset + moff,
                                        ap=[msk_sb.ap[0], [0, 2], [1, 256]])
                                    nc.vector.tensor_mul(p, p, mask_b)
                                cur.append((kt, g, p))
                        # AV for the PREVIOUS batch runs while this batch's
                        # mask+exp are in flight
                        if prev is not None:
                            for pkt, g, p in prev:
                                av1(pkt, g, p)
                            if filler is not None:
                                filler()
                        prev = cur
                    for pkt, g, p in prev:
                        av1(pkt, g, p)
                    # normalize + evacuate; og rows 64-127 hold the
                    # denominator replicated across 64 partitions
                    for g in range(2):
                        rec = rcp.tile([64, 512], F32, tag="rec",
                                       name=f"r{j}{hf}{g}")
                        nc.vector.reciprocal(rec, og[g][64:128, :])
                        oc = 2 * g + hf
                        for r in range(2):   # rep 2hf (even), 2hf+1 (odd)
                            dst = otn_sb if r == 0 else otn2_sb
                            nc.vector.tensor_mul(
                                dst[0:64, ds(oc * 2048 + j * 256, 256)],
                                og[g][0:64, r * 256:(r + 1) * 256],
                                rec[:, r * 256:(r + 1) * 256])
                    if filler is not None:
                        filler()

            def stage_otn(th):
                # odd-rep otn rows (otn2, partitions 0-63) -> otn rows 64-127
                # for this token half
                for oc in range(4):
                    nc.sync.dma_start(
                        out=otn_sb[64:128, ds(oc * 2048 + th * 1024, 1024)],
                        in_=otn2_sb[0:64, ds(oc * 2048 + th * 1024, 1024)])

            def c_block(th, oc, tag="pa", bufs=2):
                for qh in range(2):
                    yt = ps.tile([128, 512], F32, tag=tag, bufs=bufs,
                                 name=f"yt{th}_{oc}_{qh}")
                    for ic in range(4):
                        nc.tensor.matmul(
                            yt,
                            wo_sb[:, ic * 2048 + oc * 128:
                                  ic * 2048 + (oc + 1) * 128],
                            otn_sb[:, ds(ic * 2048 + th * 1024 + qh * 512,
                                         512)],
                            start=(ic == 0), stop=(ic == 3))
                    yt_sb = ytp.tile([128, 512], BF16, tag="ytsb",
                                     name=f"ytsb{th}_{oc}_{qh}")
                    nc.vector.tensor_copy(yt_sb, yt)
                    nc.sync.dma_start(
                        out=YT[oc * 128:(oc + 1) * 128,
                               ds(th * 1024 + qh * 512, 512)],
                        in_=yt_sb)

            def body(_rep):
                # ---------------- input DMA ----------------
                # host pre-arranges every input into its SBUF layout; x/wq/
                # wk/wv stream in 4 ck-groups so phase A can start on group 0
                # while later groups are in flight; wo (phase C only) last.
                nc.sync.dma_start(out=msk_sb, in_=MSK4)
                for cg in range(4):
                    nc.sync.dma_start(out=x_sb[:, ds(cg * 8192, 8192)],
                                      in_=xT[:, ds(cg * 8192, 8192)])
                    nc.sync.dma_start(out=wq_sb[:, ds(cg * 2048, 2048)],
                                      in_=WqT[:, ds(cg * 2048, 2048)])
                    nc.sync.dma_start(out=wk_sb[:, ds(cg * 512, 512)],
                                      in_=WkT[:, ds(cg * 512, 512)])
                    nc.sync.dma_start(out=wv_sb[:, ds(cg * 512, 512)],
                                      in_=WvT[:, ds(cg * 512, 512)])
                nc.sync.dma_start(out=wo_sb, in_=WoT)

                # ---------------- staggered A / B / C pipeline -------------
                # A(tb) feeds B(2tb), B(2tb+1); emit A one block ahead so its
                # matmuls fill PE slack under the previous B pair's exps.
                phase_a0()
                phase_b(0)
                phase_b(1)
                phase_a(1)
                phase_b(2)
                phase_b(3)
                stage_otn(0)
                phase_a(2)
                phase_b(4)
                phase_b(5)
                phase_a(3)
                # C token-half 0 interleaves into B(6,7)'s exp-bound batches
                pending = [(0, oc) for oc in range(16)]

                def filler():
                    if pending:
                        c_block(*pending.pop(0))

                phase_b(6, filler=filler)
                phase_b(7, filler=filler)
                while pending:
                    c_block(*pending.pop(0))
                stage_otn(1)
                for oc in range(16):
                    # B is done: run the tail on the idle 4-deep og bank set
                    # so PE isn't gated on evacuation latency
                    c_block(1, oc, tag="og", bufs=4)

            loop(loop_n, body)

    nc.compile()
    return nc


def _get_nc():
    if "nc" not in _CACHE:
        _CACHE["nc"] = _build_nc()
    return _CACHE["nc"]


# --------------------------------------------------------------------------
# host wrapper
# --------------------------------------------------------------------------
def _bf16(a):
    import ml_dtypes
    return np.ascontiguousarray(np.asarray(a).astype(ml_dtypes.bfloat16))


def _make_mask() -> np.ndarray:
    """4 multiplicative 0/1 blocks of [128, 256] (broadcast over reps):
    block 0: all-pass; 1: diag kt==2j; 2: diag kt==2j+1; 3: all-blocked."""
    ki = np.arange(128)[:, None]
    qi = np.arange(256)[None, :]
    o = np.ones((128, 256), np.float32)
    m0 = np.where(ki <= qi, 1.0, 0.0).astype(np.float32)
    m1 = np.where(128 + ki <= qi, 1.0, 0.0).astype(np.float32)
    mf = np.zeros((128, 256), np.float32)
    return np.concatenate([o, m0, m1, mf], axis=1)  # [128, 1024]


def _core_inputs(x, Wq, Wk, Wv, Wo, c, mask):
    b, hb = c // 4, c % 4
    xT_c = np.ascontiguousarray(x[b].T)
    # interleave q heads: chunk qc = [g0 rep qc (64) | g1 rep qc (64)]
    g0, g1 = 2 * hb, 2 * hb + 1
    cols = []
    for qc in range(NREP):
        cols.append(Wq[g0 * 256 + qc * 64: g0 * 256 + (qc + 1) * 64])
        cols.append(Wq[g1 * 256 + qc * 64: g1 * 256 + (qc + 1) * 64])
    WqT_c = np.ascontiguousarray(np.concatenate(cols, axis=0).T)
    WkT_c = np.ascontiguousarray(Wk[128 * hb:128 * (hb + 1)].T)
    WvT_c = np.ascontiguousarray(Wv[128 * hb:128 * (hb + 1)].T)
    WoT_c = np.ascontiguousarray(Wo[:, 512 * hb:512 * (hb + 1)].T)
    def _sb(a, nchunk):    # [nchunk*128, w] -> [128, nchunk*w] (ck-major cols)
        n = a.shape[0] // 128
        assert n == nchunk
        return a.reshape(n, 128, a.shape[1]).transpose(1, 0, 2).reshape(
            128, n * a.shape[1])
    return {"xT": _bf16(_sb(xT_c, 16)), "WqT": _bf16(_sb(WqT_c, 16)),
            "WkT": _bf16(_sb(WkT_c, 16)), "WvT": _bf16(_sb(WvT_c, 16)),
            "WoT": _bf16(_sb(WoT_c, 4)), "MSK4": _bf16(mask)}


def kernel(x, Wq, Wk, Wv, Wo, _trace=False, _trace_kwargs=None):
    from concourse import bass_utils

    x = np.asarray(x, dtype=np.float32)
    Wq = np.asarray(Wq, dtype=np.float32)
    Wk = np.asarray(Wk, dtype=np.float32)
    Wv = np.asarray(Wv, dtype=np.float32)
    Wo = np.asarray(Wo, dtype=np.float32)

    nc = _get_nc()
    mask = _make_mask()
    in_maps = [_core_inputs(x, Wq, Wk, Wv, Wo, c, mask) for c in range(8)]

    res = None
    last_exc = None
    for _attempt in range(3):
        try:
            res = bass_utils.run_bass_kernel_spmd(
                nc, in_maps, core_ids=list(range(8)),
                trace=_trace, **(_trace_kwargs or {}))
            break
        except Exception as e:  # transient device wedges happen; retry
            last_exc = e
    if res is None:
        raise last_exc

    Y = np.zeros((B, T, D_MODEL), dtype=np.float32)
    for c in range(8):
        Y[c // 4] += res.results[c]["YT"].T.astype(np.float32)
    if _trace:
        _CACHE["last_result"] = res
    return Y
